# revision 78
# baseline (speedup 1.0000x reference)
"""AttentionFreeTransformer on 8 trn2 NeuronCores — all-fp8 matmuls.

Sharding: batch b -> core pair (2b, 2b+1); each core owns half the sequence
(T = S/2 tokens). The AFT cumsum couples cores only through per-channel
running totals, exchanged as two tiny pair-wise AllReduces.

Phase A runs mm1 in K -> V -> Q order with the K and Q c-tile loops split by
token half (ncb) so inv-rms rows are ready at half-loop boundaries. The
cumsum scans (DVE, in place) run inside the V/Q matmul windows; exp rides
ACT with accum_out giving the w totals for free; kv totals come from the kv
scan's last column. The w-totals AllReduce fires as the V loop starts, the
kv-totals one at V-loop end, so both carries land inside the Q window.
Carries fold into existing op slots: tensor_scalar's second scalar (wc
shift + scale) and the y STT's scalar slot (kv carry). Collective masking
runs on GpSimd (kn/qn multiplies too) so the in-order DVE queue never
stalls on a carry. rms row chains run fully on ACT (sqrt fused with
scale+eps, reciprocal via exp(-ln)). mm2 starts essentially at Q-loop end:
ncb-outer over f-tiles (weights streamed twice), then mm3 with residual.

All three matmuls in fp8 (e4m3) DoubleRow. rms(x) applied on host; weights
pre-scaled by 32 (fp8 normal range); y carries scale YS^2/WQS = 8, h
carries 32*h; every inverse scale folds into an op that existed anyway.
"""

import os
import sys

for _p in ("/opt/trn_rl_repo", "/root/.axon_site/_ro/trn_rl_repo"):
    if os.path.isdir(_p) and _p not in sys.path:
        sys.path.append(_p)

import numpy as np
import ml_dtypes

import concourse.bass as bass
import concourse.mybir as mybir
import concourse.tile as tile
from concourse import bacc
from concourse.bass_utils import run_bass_kernel_spmd

F32 = mybir.dt.float32
BF16 = mybir.dt.bfloat16
FP8 = mybir.dt.float8e4
AF = mybir.ActivationFunctionType
ALU = mybir.AluOpType
DR = mybir.MatmulPerfMode.DoubleRow

WQS = 32.0     # host pre-scale on w_qkv (fp8 normal range)
MSCALE = 32.0  # host pre-scale on w_swiglu / w_out
YS = 16.0      # scale on the V drain (wv = YS*w*v)
SY = YS * YS / WQS  # net scale carried by y into mm2

EPS = 1.1920929e-07  # torch rms_norm eps=None -> finfo(float32).eps
P = 128
N_CORES = 8
DEBUG_TAPS = False


def build_nc(B, S, D, DFF):
    """Build the single-core SPMD program (same on all 8 cores)."""
    assert B * 2 == N_CORES
    T = S // 2             # tokens per core
    TD = D // P            # d-chunks (contraction)
    FU = DFF // P          # u f-tiles (same count for g)
    FQ = FU // 4
    F2 = FU // 2           # w_out half-tile split (SBUF)
    TC = min(512, T)       # token chunk for matmul free dim
    NT = T // TC           # token chunks
    DC = min(512, D)       # matmul3 d-chunk
    ND = D // DC
    NTT = T // P           # matmul3 token tiles
    assert T % P == 0 and D % P == 0 and DFF % P == 0 and TD % 2 == 0
    assert NT == 2

    nc = bacc.Bacc("TRN2", target_bir_lowering=False, debug=False,
                   num_devices=N_CORES)

    xT_d = nc.dram_tensor("xT", [P, TD * T], FP8, kind="ExternalInput")
    xres_d = nc.dram_tensor("xres", [T, D], F32, kind="ExternalInput")
    wq_d = nc.dram_tensor("wq", [3 * D, TD * P], FP8, kind="ExternalInput")
    ws_d = nc.dram_tensor("ws", [FU * P, 2 * TD * P], FP8,
                          kind="ExternalInput")
    wo_d = nc.dram_tensor("wo", [ND * P, FU * DC], FP8, kind="ExternalInput")
    mask_d = nc.dram_tensor("mask", [1, 2], F32, kind="ExternalInput")
    out_d = nc.dram_tensor("out", [T, D], F32, kind="ExternalOutput")

    if DEBUG_TAPS:
        dbg_w = nc.dram_tensor("dbg_w", [P, TD * T], BF16,
                               kind="ExternalOutput")
        dbg_wc = nc.dram_tensor("dbg_wc", [P, TD * T], BF16,
                                kind="ExternalOutput")
        dbg_kv = nc.dram_tensor("dbg_kv", [P, TD * T], BF16,
                                kind="ExternalOutput")
        dbg_cw = nc.dram_tensor("dbg_cw", [P, TD], F32,
                                kind="ExternalOutput")
        dbg_ck = nc.dram_tensor("dbg_ck", [P, TD], F32,
                                kind="ExternalOutput")
        dbg_y = nc.dram_tensor("dbg_y", [P, 2 * T * (TD // 2)], BF16,
                               kind="ExternalOutput")
        dbg_sig = nc.dram_tensor("dbg_sig", [P, TD * T], BF16,
                                 kind="ExternalOutput")

    ccw_in = nc.dram_tensor("ccw_in", [P, TD], F32)
    ccw_out = nc.dram_tensor("ccw_out", [P, TD], F32)
    cck_in = nc.dram_tensor("cck_in", [P, TD], F32)
    cck_out = nc.dram_tensor("cck_out", [P, TD], F32)

    xT_v = xT_d.ap().rearrange("p (o t) -> p o t", o=TD)        # [P,TD,T]
    wq_v = wq_d.ap().rearrange("(n p) (o c) -> p n o c", p=P, o=TD)
    ws_v = ws_d.ap().rearrange("(n p) (u o c) -> p n u o c", p=P, u=2, o=TD)
    wo_v = wo_d.ap().rearrange("(n p) (o j) -> p n o j", p=P, o=FU)
    xr_v = xres_d.ap().rearrange("(o p) d -> p o d", p=P)       # [P,T//P,D]
    out_v = out_d.ap().rearrange("(o p) d -> p o d", p=P)

    groups = [[2 * b, 2 * b + 1] for b in range(B)]

    with tile.TileContext(nc) as tc:
        # LIFO pool stack: everything above poolY is released at phase-B
        # start (in reverse creation order) so mm2/mm3 pools reuse the space
        persist = tc.alloc_tile_pool(name="persist", bufs=1)
        poolY = tc.alloc_tile_pool(name="poolY", bufs=1)
        psA = tc.alloc_tile_pool(name="psA", bufs=1, space="PSUM")
        poolX = tc.alloc_tile_pool(name="poolX", bufs=1)
        poolK = tc.alloc_tile_pool(name="poolK", bufs=1)
        poolWC = tc.alloc_tile_pool(name="poolWC", bufs=1)
        rotA = tc.alloc_tile_pool(name="rotA", bufs=1)
        wqS = tc.alloc_tile_pool(name="wqS", bufs=1)
        poolWV = tc.alloc_tile_pool(name="poolWV", bufs=1)
        poolQ = tc.alloc_tile_pool(name="poolQ", bufs=1)

        # ---- startup DMAs: first weight tile, then x in small chunks ----
        wq0 = wqS.tile([P, TD, P], FP8, name="wq_t", tag="wq", bufs=3)
        nc.sync.dma_start(wq0[:], wq_v[:, TD])      # first K c-tile weights
        xT_sb = poolX.tile([P, TD, T], FP8, name="xT_sb")
        # h0 token-half first: the K-ncb0 matmuls only read tokens [0,TC),
        # so the PE starts after 1MB instead of 2MB; h1 chunks interleave
        # into the K-ncb0 weight stream below (same ring, same bytes)
        for i in range(0, TD, 2):
            nc.sync.dma_start(xT_sb[:, i:i + 2, 0:TC],
                              xT_v[:, i:i + 2, 0:TC])

        ones_col = persist.tile([P, 1], BF16, name="ones_col")
        nc.vector.memset(ones_col[:], 1.0)
        mask_rep = persist.tile([P, 2], F32, name="mask_rep")
        nc.sync.dma_start(mask_rep[:], mask_d.ap().to_broadcast((P, 2)))
        # [P,TD] masks/eps for the Pool-side collective masking
        m16_own = persist.tile([P, TD], F32, name="m16_own")
        nc.vector.tensor_copy(m16_own[:],
                              mask_rep[:, 0:1].to_broadcast((P, TD)))
        m16_oth = persist.tile([P, TD], F32, name="m16_oth")
        nc.vector.tensor_copy(m16_oth[:],
                              mask_rep[:, 1:2].to_broadcast((P, TD)))
        eps16 = persist.tile([P, TD], F32, name="eps16")
        nc.vector.memset(eps16[:], 1e-6)
        eps_row = persist.tile([P, 1], F32, name="eps_row")
        nc.vector.memset(eps_row[:], EPS)

        kT = poolK.tile([P, TD, T], BF16, name="kT")     # k -> kn -> w
        wc = poolWC.tile([P, TD, T], BF16, name="wc")    # cumsum of w
        wv = poolWV.tile([P, TD, T], BF16, name="wv")    # v -> w*v -> kvcum
        qT = poolQ.tile([P, TD, T], BF16, name="qT")     # q -> qn -> sig
        y_pairs = [poolY.tile([P, 2, T], FP8, name=f"y_{cp}", tag="ytile",
                              bufs=TD // 2) for cp in range(TD // 2)]

        tw = persist.tile([P, 2 * TD], F32, name="tw")   # exp accum halves
        totw = persist.tile([P, TD], F32, name="totw")
        ccw_sb = persist.tile([P, TD], F32, name="ccw_sb")
        cck_sb = persist.tile([P, TD], F32, name="cck_sb")
        carry_w = persist.tile([P, TD], F32, name="carry_w")
        carry_kv = persist.tile([P, TD], F32, name="carry_kv")

        def mm1_ci(proj, ncb, ci, dst, row, wq_t=None):
            """PE work for one (projection, token-half, c-tile)."""
            tsl = slice(ncb * TC, (ncb + 1) * TC)
            ct = proj * TD + ci
            if wq_t is None:
                wq_t = wqS.tile([P, TD, P], FP8, name="wq_t", tag="wq",
                                bufs=3)
                nc.sync.dma_start(wq_t[:], wq_v[:, ct])
            ps = psA.tile([P, TC], F32, name="mm1", tag="mm1", bufs=4,
                          space="PSUM")
            for dk in range(0, TD, 2):
                nc.tensor.matmul(
                    ps[:], wq_t[:, dk:dk + 2, :], xT_sb[:, dk:dk + 2, tsl],
                    start=(dk == 0), stop=(dk == TD - 2), perf_mode=DR)
            if row is not None:
                sqt = rotA.tile([P, TC], BF16, name="sqt", tag="sqt",
                                bufs=2)
                nc.scalar.square(sqt[:], ps[:])
                nc.tensor.matmul(row[0:1, :], ones_col[:], sqt[:],
                                 start=(ci == 0), stop=(ci == TD - 1))
            dscale = (YS / WQS) if proj == 2 else 1.0 / WQS
            nc.scalar.activation(dst[:, ci, tsl], ps[:], AF.Copy,
                                 scale=dscale)

        def row_chain_dve(row, label):
            """K-side inv-rms chain on DVE + one ACT sqrt: no Ln/Exp table
            loads stalling the drain stream at section boundaries."""
            a_row = rotA.tile([1, TC], F32, name=f"a_{label}", tag="row",
                              bufs=2)
            nc.vector.tensor_scalar(a_row[:], row[0:1, :],
                                    1.0 / (WQS * WQS * D), EPS,
                                    ALU.mult, ALU.add)
            s_row = rotA.tile([1, TC], F32, name=f"s_{label}", tag="row2",
                              bufs=2)
            nc.scalar.sqrt(s_row[:], a_row[:])
            i_row = rotA.tile([1, TC], F32, name=f"i_{label}", tag="row3",
                              bufs=2)
            nc.vector.reciprocal_approx_fast(i_row[:], s_row[:])
            ib_row = rotA.tile([1, TC], BF16, name=f"ib_{label}",
                               tag="rowb", bufs=2)
            nc.vector.tensor_copy(ib_row[:], i_row[:])
            rep = rotA.tile([P, TC], BF16, name=f"rep_{label}", tag="rep",
                            bufs=2)
            nc.gpsimd.partition_broadcast(rep[:], ib_row[:])
            return rep

        def row_chain(row, label):
            """[P,TC] psum row 0 = ssq of the WQS-scaled raw projection ->
            inv-rms replicated [P,TC] bf16. All ACT + gpsimd."""
            s_row = rotA.tile([1, TC], F32, name=f"s_{label}", tag="row",
                              bufs=2)
            nc.scalar.activation(s_row[:], row[0:1, :], AF.Sqrt,
                                 scale=1.0 / (WQS * WQS * D),
                                 bias=eps_row[0:1, 0:1])
            l_row = rotA.tile([1, TC], F32, name=f"l_{label}", tag="row2",
                              bufs=2)
            nc.scalar.activation(l_row[:], s_row[:], AF.Ln)
            ib_row = rotA.tile([1, TC], BF16, name=f"i_{label}", tag="rowb",
                               bufs=2)
            nc.scalar.activation(ib_row[:], l_row[:], AF.Exp, scale=-1.0)
            rep = rotA.tile([P, TC], BF16, name=f"rep_{label}", tag="rep",
                            bufs=2)
            nc.gpsimd.partition_broadcast(rep[:], ib_row[:])
            return rep

        def ssq_row(tag):
            return psA.tile([P, TC], F32, name=f"psr_{tag}", tag="psr",
                            bufs=2, space="PSUM")

        h0 = slice(0, TC)
        h1 = slice(TC, 2 * TC)

        # ---- K ncb0 ----
        k_row0 = ssq_row("k0")
        for ci in range(TD):
            mm1_ci(1, 0, ci, kT, k_row0, wq_t=wq0 if ci == 0 else None)
            if ci % 2 == 0:
                nc.sync.dma_start(xT_sb[:, ci:ci + 2, TC:T],
                                  xT_v[:, ci:ci + 2, TC:T])
        rep_k0 = row_chain_dve(k_row0, "k0")

        # ---- K ncb1; kn0/exp0/scan_w0 ride along per c-tile ----
        k_row1 = ssq_row("k1")
        for ci in range(TD):
            mm1_ci(1, 1, ci, kT, k_row1)
            nc.gpsimd.tensor_tensor(kT[:, ci, h0], kT[:, ci, h0],
                                    rep_k0[:], ALU.mult)
            nc.scalar.activation(kT[:, ci, h0], kT[:, ci, h0], AF.Exp,
                                 accum_out=tw[:, ci:ci + 1])
            nc.vector.tensor_tensor_scan(
                wc[:, ci, h0], kT[:, ci, h0], kT[:, ci, h0], 0.0,
                ALU.add, ALU.bypass)
        rep_k1 = row_chain_dve(k_row1, "k1")
        # kn1 on Pool (executes while the V loop streams)
        for ci in range(TD):
            nc.gpsimd.tensor_tensor(kT[:, ci, h1], kT[:, ci, h1],
                                    rep_k1[:], ALU.mult)

        # ---- V loop; exp1 paced 2-per-ct; wv/kv-scan/kv-total on DVE ----
        for ci in range(TD):
            ct = 2 * TD + ci
            wq_t = wqS.tile([P, TD, P], FP8, name="wq_t", tag="wq", bufs=3)
            nc.sync.dma_start(wq_t[:], wq_v[:, ct])
            for ncb in range(NT):
                tsl = slice(ncb * TC, (ncb + 1) * TC)
                ps = psA.tile([P, TC], F32, name="mm1", tag="mm1", bufs=4,
                              space="PSUM")
                for dk in range(0, TD, 2):
                    nc.tensor.matmul(
                        ps[:], wq_t[:, dk:dk + 2, :],
                        xT_sb[:, dk:dk + 2, tsl],
                        start=(dk == 0), stop=(dk == TD - 2), perf_mode=DR)
                nc.scalar.activation(wv[:, ci, tsl], ps[:], AF.Copy,
                                     scale=YS / WQS)
            if ci < TD // 2:
                for j in (2 * ci, 2 * ci + 1):
                    nc.scalar.activation(kT[:, j, h1], kT[:, j, h1], AF.Exp,
                                         accum_out=tw[:, TD + j:TD + j + 1])
            nc.vector.tensor_tensor(wv[:, ci, :], kT[:, ci, :],
                                    wv[:, ci, :], ALU.mult)
            nc.vector.tensor_tensor_scan(
                wv[:, ci, :], wv[:, ci, :], wv[:, ci, :], 0.0,
                ALU.add, ALU.bypass)
            # kv total = last scan column, masked for the pair exchange
            nc.vector.tensor_scalar_mul(cck_sb[:, ci:ci + 1],
                                        wv[:, ci, T - 1:T],
                                        mask_rep[:, 0:1])

        # w totals -> masked (Pool, so DVE never stalls) -> AllReduce #1
        nc.gpsimd.tensor_tensor(totw[:], tw[:, 0:TD], tw[:, TD:2 * TD],
                                ALU.add)
        nc.gpsimd.tensor_tensor(ccw_sb[:], totw[:], m16_own[:], ALU.mult)
        nc.sync.dma_start(ccw_in.ap(), ccw_sb[:])
        nc.gpsimd.collective_compute(
            "AllReduce", ALU.add, replica_groups=groups,
            ins=[ccw_in.ap().opt()], outs=[ccw_out.ap().opt()])
        ccw_ret = persist.tile([P, TD], F32, name="ccw_ret")
        nc.sync.dma_start(ccw_ret[:], ccw_out.ap())

        # kv totals AllReduce #2
        nc.sync.dma_start(cck_in.ap(), cck_sb[:])
        nc.gpsimd.collective_compute(
            "AllReduce", ALU.add, replica_groups=groups,
            ins=[cck_in.ap().opt()], outs=[cck_out.ap().opt()])
        cck_ret = persist.tile([P, TD], F32, name="cck_ret")
        nc.sync.dma_start(cck_ret[:], cck_out.ap())

        # carry maskings on Pool: carry_w = ret*mask + 1e-6, carry_kv = ret*mask
        nc.gpsimd.tensor_tensor(carry_w[:], ccw_ret[:], m16_oth[:],
                                ALU.mult)
        nc.gpsimd.tensor_tensor(carry_w[:], carry_w[:], eps16[:], ALU.add)
        nc.gpsimd.tensor_tensor(carry_kv[:], cck_ret[:], m16_oth[:],
                                ALU.mult)

        # ---- scan_w1 (chained from h0), then the carry midstream ----
        for ci in range(TD):
            nc.vector.tensor_tensor_scan(
                wc[:, ci, h1], kT[:, ci, h1], kT[:, ci, h1],
                wc[:, ci, TC - 1:TC], ALU.add, ALU.bypass)

        if DEBUG_TAPS:
            _r = "p (o t) -> p o t"
            nc.sync.dma_start(dbg_w.ap().rearrange(_r, o=TD), kT[:])
            nc.sync.dma_start(dbg_wc.ap().rearrange(_r, o=TD), wc[:])
            nc.sync.dma_start(dbg_kv.ap().rearrange(_r, o=TD), wv[:])
            nc.sync.dma_start(dbg_cw.ap(), carry_w[:])
            nc.sync.dma_start(dbg_ck.ap(), carry_kv[:])

        # wcs = (wc + carry_w) * WQS/YS ; rcp = 1/wcs ; y = (kvc+ckv)*rcp
        rcps = {}

        def emit_ypre(ci):
            nc.vector.scalar_tensor_tensor(
                y_pairs[ci // 2][:, ci % 2, :], wv[:, ci, :],
                carry_kv[:, ci:ci + 1], rcps.pop(ci)[:], ALU.add, ALU.mult)

        for ci in range(TD):
            wcs = rotA.tile([P, T], F32, name="wcs", tag="wcs", bufs=2)
            nc.vector.tensor_scalar(wcs[:], wc[:, ci, :],
                                    carry_w[:, ci:ci + 1], WQS / YS,
                                    ALU.add, ALU.mult)
            rcp = rotA.tile([P, T], F32, name="rcp", tag="rcp", bufs=3)
            nc.vector.reciprocal_approx_fast(rcp[:], wcs[:])
            rcps[ci] = rcp
            if ci >= 2:
                emit_ypre(ci - 2)
        for ci in range(TD - 2, TD):
            emit_ypre(ci)

        if DEBUG_TAPS:
            dbg_y_v = dbg_y.ap().rearrange("p (c u t) -> p c u t",
                                           c=TD // 2, u=2)
            for cp in range(TD // 2):
                ycvt = poolWC.tile([P, 2, T], BF16, name="ycvt", tag="ycvt",
                                   bufs=1)
                nc.vector.tensor_copy(ycvt[:], y_pairs[cp][:])
                nc.sync.dma_start(dbg_y_v[:, cp], ycvt[:])

        # ---- Q loop (both halves), then the gate streams ----
        q_row0 = ssq_row("q0")
        for ci in range(TD):
            mm1_ci(0, 0, ci, qT, q_row0)
        rep_q0 = row_chain(q_row0, "q0")
        q_row1 = ssq_row("q1")
        for ci in range(TD):
            mm1_ci(0, 1, ci, qT, q_row1)
        rep_q1 = row_chain(q_row1, "q1")

        for ncb, rep_q in ((0, rep_q0), (1, rep_q1)):
            tsl = h0 if ncb == 0 else h1
            for ci in range(TD):
                nc.gpsimd.tensor_tensor(qT[:, ci, tsl], qT[:, ci, tsl],
                                        rep_q[:], ALU.mult)
                nc.scalar.activation(qT[:, ci, tsl], qT[:, ci, tsl],
                                     AF.Sigmoid)
                nc.vector.tensor_tensor(
                    y_pairs[ci // 2][:, ci % 2, tsl],
                    y_pairs[ci // 2][:, ci % 2, tsl],
                    qT[:, ci, tsl], ALU.mult)

        if DEBUG_TAPS:
            nc.sync.dma_start(dbg_sig.ap().rearrange("p (o t) -> p o t",
                                                     o=TD), qT[:])

        poolQ.release()
        poolWV.release()
        wqS.release()
        rotA.release()
        poolWC.release()
        poolK.release()
        poolX.release()
        psA.release()

        # ---- matmul2: uv^T, h^T = u*silu(g); ncb-outer, no prefix ----
        poolB = tc.alloc_tile_pool(name="poolB", bufs=1)
        psB = tc.alloc_tile_pool(name="psB", bufs=1, space="PSUM")

        hT_q = [poolB.tile([P, FQ, T], FP8, name=f"hT_{i}")
                for i in range(4)]
        dsc = 1.0 / (SY * MSCALE)

        for ncb in range(NT):
            tsl = slice(ncb * TC, (ncb + 1) * TC)
            for fj in range(FU):
                wsg_t = poolB.tile([P, 2, TD, P], FP8, name="wsg",
                                   tag="ws", bufs=3)
                nc.sync.dma_start(wsg_t[:], ws_v[:, fj])
                psu = psB.tile([P, TC], F32, name="psu", tag="mm2", bufs=4,
                               space="PSUM")
                psg = psB.tile([P, TC], F32, name="psg", tag="mm2", bufs=4,
                               space="PSUM")
                for dk in range(0, TD, 2):
                    nc.tensor.matmul(
                        psu[:], wsg_t[:, 0, dk:dk + 2, :],
                        y_pairs[dk // 2][:, :, tsl],
                        start=(dk == 0), stop=(dk == TD - 2), perf_mode=DR)
                for dk in range(0, TD, 2):
                    nc.tensor.matmul(
                        psg[:], wsg_t[:, 1, dk:dk + 2, :],
                        y_pairs[dk // 2][:, :, tsl],
                        start=(dk == 0), stop=(dk == TD - 2), perf_mode=DR)
                sg = poolB.tile([P, TC], BF16, name="sg", tag="sg", bufs=4)
                nc.scalar.activation(sg[:], psg[:], AF.Silu, scale=dsc)
                nc.vector.scalar_tensor_tensor(
                    hT_q[fj // FQ][:, fj % FQ, tsl], psu[:], 1.0 / SY,
                    sg[:], ALU.mult, ALU.mult)

        # ---- matmul3 (+residual) ----
        poolC = tc.alloc_tile_pool(name="poolC", bufs=1)

        def hT_pair(kk, tt):
            quarter = hT_q[kk // FQ]
            m = kk % FQ
            return quarter[:, m:m + 2, tt * P:(tt + 1) * P]

        for dc in range(ND):
            dsl = slice(dc * DC, (dc + 1) * DC)
            wo_t = [poolC.tile([P, F2, DC], FP8, name=f"wo_{i}",
                               tag=f"wo{i}", bufs=2) for i in range(2)]
            for i in range(2):
                nc.sync.dma_start(wo_t[i][:],
                                  wo_v[:, dc, i * F2:(i + 1) * F2, :])
            for tt in range(NTT):
                ps3 = psB.tile([P, DC], F32, name="ps3", tag="mm3", bufs=3,
                               space="PSUM")
                for kk in range(0, FU, 2):
                    nc.tensor.matmul(
                        ps3[:], hT_pair(kk, tt),
                        wo_t[kk // F2][:, (kk % F2):(kk % F2) + 2, :],
                        start=(kk == 0), stop=(kk == FU - 2), perf_mode=DR)
                xr_t = poolC.tile([P, DC], F32, name="xr", tag="xr", bufs=3)
                nc.sync.dma_start(xr_t[:], xr_v[:, tt, dsl])
                o_t = poolC.tile([P, DC], F32, name="ot", tag="ot", bufs=3)
                nc.vector.scalar_tensor_tensor(
                    o_t[:], ps3[:], 1.0 / (MSCALE * MSCALE),
                    xr_t[:], ALU.mult, ALU.add)
                nc.sync.dma_start(out_v[:, tt, dsl], o_t[:])

        poolC.release()
        poolB.release()
        psB.release()
        poolY.release()
        persist.release()

    nc.compile()
    return nc


_NC_CACHE = {}


def _get_nc(B, S, D, DFF):
    key = (B, S, D, DFF)
    if key not in _NC_CACHE:
        _NC_CACHE[key] = build_nc(B, S, D, DFF)
    return _NC_CACHE[key]


def make_in_maps(x, w_qkv, w_swiglu, w_out):
    B, S, D = x.shape
    DFF = w_out.shape[1]
    T = S // 2
    TD = D // P
    NC3 = 3 * D // P
    FU = DFF // P
    DC = min(512, D)
    ND = D // DC
    f8 = ml_dtypes.float8_e4m3

    # weights: fp8 with pre-scales, partition-contiguous flat layouts
    wqT = (w_qkv.T * WQS).astype(f8)                       # [D, 3D]
    wq_arr = np.ascontiguousarray(
        wqT.reshape(TD, P, NC3, P).transpose(2, 1, 0, 3)
    ).reshape(NC3 * P, TD * P)
    wsT = (w_swiglu.T * MSCALE).astype(f8)                 # [D, 2DFF]
    ws_arr = np.ascontiguousarray(
        wsT.reshape(TD, P, 2, FU, P).transpose(3, 1, 2, 0, 4)
    ).reshape(FU * P, 2 * TD * P)
    woT = (w_out.T * MSCALE).astype(f8)                    # [DFF, D]
    wo_arr = np.ascontiguousarray(
        woT.reshape(FU, P, ND, DC).transpose(2, 1, 0, 3)
    ).reshape(ND * P, FU * DC)

    # host-side rms norm of x; the device sees pre-normalized fp8 x
    xn = x * (1.0 / np.sqrt((x * x).mean(axis=2, keepdims=True) + EPS))
    xn8 = xn.astype(f8)                                    # [B, S, D]

    in_maps = []
    for c in range(N_CORES):
        b, h = divmod(c, 2)
        xc8 = xn8[b, h * T:(h + 1) * T]                    # [T, D] fp8
        xT8 = np.ascontiguousarray(
            xc8.T.reshape(TD, P, T).transpose(1, 0, 2)).reshape(P, TD * T)
        in_maps.append({
            "xT": xT8,
            "xres": np.ascontiguousarray(x[b, h * T:(h + 1) * T],
                                         dtype=np.float32),
            "wq": wq_arr,
            "ws": ws_arr,
            "wo": wo_arr,
            "mask": np.array([[1.0 - h, float(h)]], np.float32),
        })
    return in_maps


def assemble_out(results, B, S, D):
    T = S // 2
    out = np.empty((B, S, D), np.float32)
    for c in range(N_CORES):
        b, h = divmod(c, 2)
        out[b, h * T:(h + 1) * T] = results[c]["out"]
    return out


def kernel(x, w_qkv, w_swiglu, w_out):
    x = np.asarray(x, dtype=np.float32)
    w_qkv = np.asarray(w_qkv, dtype=np.float32)
    w_swiglu = np.asarray(w_swiglu, dtype=np.float32)
    w_out = np.asarray(w_out, dtype=np.float32)
    B, S, D = x.shape
    DFF = w_out.shape[1]
    nc = _get_nc(B, S, D, DFF)
    in_maps = make_in_maps(x, w_qkv, w_swiglu, w_out)
    res = run_bass_kernel_spmd(nc, in_maps, core_ids=list(range(N_CORES)))
    return assemble_out(res.results, B, S, D)
